# revision 4
# baseline (speedup 1.0000x reference)
"""Trainium2 Bass kernel for nn_BBPMAssociativeModel.

Model: per-batch associative memory — pairs (key, value-token) from the
input sequence are scatter-added into a 8192-slot memory via 4 hash
probes, the memory is read back at the query token's 4 probe slots,
and the mean read vector goes through a [D, V] classifier.

Algebraic collapse: the memory is never materialized, and the read
vector r is computed ON HOST (it is a tiny [B, D] combination of a few
embedding rows selected by integer hash matches — ~2 MFLOP):
    r_b = sum_p (m_{b,p} / K) * emb_table[x[b, 2p+1]]
The device does only the memory-bound classifier matmul
    logits = r @ W.T          (vocab-sharded over 8 cores)

Per-core device program (vocab shard of 4000 columns):
  - rt   [128, 128]  fp16: rT k-chunk-packed, rt[p, k*32+b] = r[b, k*128+p]/S
  - wt   [128, 16000] fp8e3 (e3m4): W shard, block-major packed
         wt[p, j*2000 + k*500 + n] = S * W[c*4000 + j*500 + n, k*128 + p]
         e3m4 halves the W stream vs fp16; with rt kept fp16 the logits
         rel-err is ~1.3e-2 (gate 2e-2). S=128 maps W into e3m4's
         normal range.
  - out  [32, 4000]  fp16 logits shard (host upcasts, adds b)
The stream is issued as 8 DMAs alternating the two HWDGE queues, with
the final block split fine so the kernel tail starts early.
"""

import numpy as np
from contextlib import ExitStack

B, T, D, V = 32, 2048, 512, 32000
NCORES = 8
VS = V // NCORES        # 4000 vocab columns per core
NUM_SLOTS, KP = 8192, 4
SEED = np.uint32(1234)
GOLD = np.uint32(0x9E3779B9)
KC = D // 128           # 4 contraction chunks
NBW = 500               # columns per block (one PSUM bank of fp32)
NB = VS // NBW          # 8 blocks per core
BLK = KC * NBW          # 2000 free elems per block in wt layout

# W-stream dtype: "f8" = e3m4 (1 byte, rel-err ~1.3e-2),
# "f16" = float16 (2 bytes, rel-err ~3e-4).
W_MODE = "f8"
WSCALE = 128.0
NWARM = 3               # PE HAM warm-up dummy matmuls

_prog_cache = {}
LAST_RESULTS = None     # stashed BassKernelResults (for profiling in test.py)


def _mix32(h):
    h = h.astype(np.uint32, copy=False)
    h = h ^ (h >> np.uint32(16))
    h = h * np.uint32(0x85EBCA6B)
    h = h ^ (h >> np.uint32(13))
    h = h * np.uint32(0xC2B2AE35)
    h = h ^ (h >> np.uint32(16))
    return h


def _probe_slots(tok):
    hx = _mix32(tok.astype(np.uint32) ^ SEED)
    offs = np.arange(KP, dtype=np.uint32) * GOLD
    return (_mix32(hx[..., None] + offs) % np.uint32(NUM_SLOTS)).astype(np.int32)


def _split_multi_waits(nc, limit=1):
    """The nix-baked walrus rejects instructions with more than `limit`
    sem-waits ("Too many sync wait commands", CoreV3GenImpl setupSyncWait).
    Hoist extra waits onto single-wait NOPs preceding the instruction on
    the same engine (waiting earlier on the same engine is always safe)."""
    import concourse.mybir as mybir

    for fn in nc.m.functions:
        for bb in fn.blocks:
            new_insts = []
            for ins in bb.instructions:
                si = ins.sync_info
                if si is not None and len(si.on_wait) > limit:
                    waits = list(si.on_wait)
                    extra, keep = waits[:-limit], waits[-limit:]
                    for idx, w in enumerate(extra):
                        new_insts.append(mybir.InstNoOp(
                            name=f"{ins.name}-wsplit{idx}",
                            sync_info=mybir.SyncInfo(on_wait=[w], on_update=[]),
                            bass_nofuse=True,
                            engine=ins.engine,
                        ))
                    ins.sync_info = mybir.SyncInfo(
                        on_wait=keep, on_update=list(si.on_update))
                new_insts.append(ins)
            bb.instructions[:] = new_insts


def _strip_entry_barrier(nc):
    """Remove the entry-BB all-engine boot barrier and the const-tile
    memsets (walrus flags those consts as having no readers). The barrier
    only serializes engine boot: every real dependency in the body is
    carried by Tile-generated semaphores, and the event-semaphore
    barrier instances are self-resetting, so the exit barriers are
    unaffected. This lets each engine (notably the DMA-trigger engines)
    start its body work as soon as it boots instead of waiting ~3us for
    the slowest engine."""
    import concourse.mybir as mybir

    def _is_barrier(ins):
        if not isinstance(ins, (mybir.InstDrain, mybir.InstEventSemaphore)):
            return False
        si = ins.sync_info
        names = [w.ant_name for w in (si.on_wait if si else [])]
        names += [getattr(u, "ant_name", "") or ""
                  for u in (si.on_update if si else [])]
        return any(n.startswith("barrier_") for n in names) or not names

    bb = nc.m.functions[0].blocks[0]
    bb.instructions[:] = [
        ins for ins in bb.instructions
        if not (isinstance(ins, mybir.InstMemset) or _is_barrier(ins))
    ]


def _build(wdt, split=True):
    import concourse.bass as bass
    import concourse.mybir as mybir
    from concourse.bass import MemorySpace
    from concourse.tile import TileContext

    f32 = mybir.dt.float32
    f16 = mybir.dt.float16
    fw = mybir.dt.float8e3 if wdt == "f8" else mybir.dt.float16

    nc = bass.Bass(monotonic_sem_count=0, enable_partition_id=False)
    rt = nc.declare_dram_parameter("rt", [128, KC * B], f16, isOutput=False)
    wt = nc.declare_dram_parameter("wt", [128, KC * VS], fw, isOutput=False)
    out = nc.declare_dram_parameter("out", [B, VS], f16, isOutput=True)

    # W-stream chunk plan: (free-dim start, length, engine idx 0=sync
    # 1=scalar). Blocks are consumed in pairs (col-group tiling), so the
    # leading pair is split at k-granularity to start the first matmuls
    # as early as possible; middle pairs are one 512 KB transfer each;
    # the trailing block is split fine (k01 | k2 | k3) so the kernel
    # tail (last matmul -> copy -> store) starts early.
    chunks = [
        (0 * BLK, 2 * NBW, 0),             # blk0 k01
        (1 * BLK, 2 * NBW, 1),             # blk1 k01
        (0 * BLK + 2 * NBW, 2 * NBW, 0),   # blk0 k23
        (1 * BLK + 2 * NBW, 2 * NBW, 1),   # blk1 k23
        (2 * BLK, 2 * BLK, 0),             # blks 2,3
        (4 * BLK, 2 * BLK, 1),             # blks 4,5
        (6 * BLK, 1 * BLK, 0),             # blk6
        (7 * BLK, 2 * NBW, 1),             # blk7 k01
        (7 * BLK + 2 * NBW, NBW, 0),       # blk7 k2
        (7 * BLK + 3 * NBW, NBW, 1),       # blk7 k3
    ]

    with TileContext(nc) as tc:
        with ExitStack() as ctx:
            const = ctx.enter_context(tc.tile_pool(name="const", bufs=1))
            rt_sb = const.tile([128, KC * B], f16)
            dumw = const.tile([128, 544], fw, name="dumw")

            wtp = ctx.enter_context(tc.tile_pool(name="wtp", bufs=len(chunks)))
            obuf = ctx.enter_context(tc.tile_pool(name="obuf", bufs=NB))
            with tc.tile_pool(name="mpsum", bufs=8, space=MemorySpace.PSUM) as mpsum:
                # rt load first on scalar (it gates the first LDWEIGHTS);
                # it is tiny so it barely delays the W stream behind it.
                nc.scalar.dma_start(rt_sb[:], rt[:])

                dma_engs = [nc.sync, nc.scalar]
                wq = []
                for (off, ln, ei) in chunks:
                    t = wtp.tile([128, ln], fw, name="wq")
                    dma_engs[ei].dma_start(t[:], wt[:, off:off + ln])
                    wq.append((off, ln, t))

                def moving(j, k):
                    g = j * BLK + k * NBW
                    for (off, ln, t) in wq:
                        if off <= g and g + NBW <= off + ln:
                            return t[:, g - off:g - off + NBW]
                    raise AssertionError("no chunk covers block")

                # PE warm-up: the HAM clock gate keeps the PE at 1.2 GHz
                # until it has seen ~3.4us of sustained matmul activity.
                # Dummy matmuls on an SBUF-garbage-free memset tile keep
                # the PE busy while the first W chunks are in flight so
                # the real matmuls run at 2.4 GHz.
                nc.vector.memset(dumw[:], 0.0)
                dps = mpsum.tile([32, 512], f32, name="ps")
                for _ in range(NWARM):
                    nc.tensor.matmul(
                        dps[:], dumw[:, :32], dumw[:, 32:544],
                        start=True, stop=True)

                # Blocks are processed in pairs on the two 32-column
                # groups of the PE array (col-group tiling): even block
                # -> col-group 0 / PSUM partitions [0:32], odd block ->
                # col-group 1 / PSUM partitions [32:64]. The two moving
                # streams run concurrently, halving the PE-serial time.
                ps_t, ob_t = [], []
                for j in range(NB):
                    ps = mpsum.tile([64, NBW], f32, name="ps")
                    ob = obuf.tile([64, NBW], f16, name="ob")
                    ps_t.append(ps)
                    ob_t.append(ob)

                def psum_ap(j):
                    lo = 32 * (j % 2)
                    return ps_t[j][lo:lo + B]

                for p in range(NB // 2):
                    for k in range(KC):
                        for j in (2 * p, 2 * p + 1):
                            nc.tensor.matmul(
                                psum_ap(j),
                                rt_sb[:, k * B:(k + 1) * B],
                                moving(j, k),
                                start=(k == 0),
                                stop=(k == KC - 1),
                                tile_position=(0, 32 * (j % 2)),
                            )

                    for j in (2 * p, 2 * p + 1):
                        lo = 32 * (j % 2)
                        ob = ob_t[j][lo:lo + B]
                        cols = slice(j * NBW, (j + 1) * NBW)
                        if j == NB - 1:
                            # Final block: halve the copy across DVE+ACT
                            # and store the halves on the two HWDGE
                            # queues so the last write receipt (which
                            # gates the kernel tail) lands sooner.
                            h = NBW // 2
                            nc.vector.tensor_copy(
                                ob[:, :h], psum_ap(j)[:, :h])
                            nc.scalar.copy(
                                ob[:, h:], psum_ap(j)[:, h:])
                            nc.sync.dma_start(
                                out[:, j * NBW:j * NBW + h], ob[:, :h])
                            nc.scalar.dma_start(
                                out[:, j * NBW + h:(j + 1) * NBW],
                                ob[:, h:])
                        else:
                            if j % 2 == 0:
                                nc.vector.tensor_copy(ob[:], psum_ap(j))
                            else:
                                nc.scalar.copy(ob[:], psum_ap(j))
                            # even-block stores on SWDGE, odd on sync
                            # (free after its W triggers)
                            if j % 2 == 0:
                                nc.gpsimd.dma_start(out[:, cols], ob[:])
                            else:
                                nc.sync.dma_start(out[:, cols], ob[:])
    if split:
        _split_multi_waits(nc)
        _strip_entry_barrier(nc)
    return nc


def _get_prog():
    if W_MODE not in _prog_cache:
        _prog_cache[W_MODE] = _build(W_MODE)
    return _prog_cache[W_MODE]


def _host_r(x, emb_table):
    """Integer hash/match preprocessing -> read vector r [B, D]."""
    ts = np.arange(0, T - 1, 2)
    ts = ts[ts + 1 < T - 1]                      # [P]
    wslots = _probe_slots(x[:, ts])              # [B, P, K]
    qslots = _probe_slots(x[:, -1])              # [B, K]
    m = (wslots[:, :, None, :] == qslots[:, None, :, None]).sum(
        axis=(2, 3), dtype=np.int32)             # [B, P]
    bs, ps = np.nonzero(m)
    r = np.zeros((B, D), np.float64)
    vtok = x[:, ts + 1]
    for bi, pi in zip(bs, ps):
        r[bi] += (m[bi, pi] / KP) * emb_table[vtok[bi, pi]].astype(np.float64)
    return r


def _pack_w(W):
    """[V, D] -> per-core block-major stream layout [NCORES, 128, KC*VS]."""
    import ml_dtypes
    np_w = ml_dtypes.float8_e3m4 if W_MODE == "f8" else np.float16
    Wq = (W.astype(np.float32) * np.float32(WSCALE)).astype(np_w)
    A = Wq.reshape(NCORES, NB, NBW, KC, 128)     # [c, j, n, k, p]
    return np.ascontiguousarray(
        A.transpose(0, 4, 1, 3, 2)).reshape(NCORES, 128, KC * VS)


def kernel(x, emb_table, W, b):
    global LAST_RESULTS
    from concourse.bass_utils import run_bass_kernel_spmd

    x = np.asarray(x)
    emb_table = np.asarray(emb_table, np.float32)
    W = np.asarray(W, np.float32)
    b = np.asarray(b, np.float32)

    r = _host_r(x, emb_table)                    # [B, D] float64
    rt = (r / WSCALE).astype(np.float16)         # fold W scale into rt
    rt_dev = np.ascontiguousarray(
        rt.T.reshape(KC, 128, B).transpose(1, 0, 2)).reshape(128, KC * B)
    wt_dev = _pack_w(W)

    nc = _get_prog()
    in_maps = [{"rt": rt_dev, "wt": wt_dev[c]} for c in range(NCORES)]

    res = None
    for attempt in range(3):
        try:
            res = run_bass_kernel_spmd(
                nc, in_maps, core_ids=list(range(NCORES)))
            break
        except Exception:
            # The axon-tunneled device occasionally reports a transient
            # NRT_EXEC_UNIT_UNRECOVERABLE on back-to-back NEFF loads;
            # a re-dispatch on the next attempt succeeds.
            if attempt == 2:
                raise
            import time
            time.sleep(2.0)
    LAST_RESULTS = res

    logits = np.empty((B, V), np.float32)
    for c in range(NCORES):
        logits[:, c * VS:(c + 1) * VS] = res.results[c]["out"]
    if np.any(b):
        logits += b[None, :]
    return logits


# revision 7
# speedup vs baseline: 1.0281x; 1.0281x over previous
"""Trainium2 Bass kernel for nn_BBPMAssociativeModel.

Model: per-batch associative memory — pairs (key, value-token) from the
input sequence are scatter-added into a 8192-slot memory via 4 hash
probes, the memory is read back at the query token's 4 probe slots,
and the mean read vector goes through a [D, V] classifier.

Algebraic collapse: the memory is never materialized, and the read
vector r is computed ON HOST (it is a tiny [B, D] combination of a few
embedding rows selected by integer hash matches — ~2 MFLOP):
    r_b = sum_p (m_{b,p} / K) * emb_table[x[b, 2p+1]]
The device does only the memory-bound classifier matmul
    logits = r @ W.T          (vocab-sharded over 8 cores)

Per-core device program (vocab shard of 4000 columns):
  - rt   [128, 128]  fp16: rT k-chunk-packed, rt[p, k*32+b] = r[b, k*128+p]/S
  - wt   [128, 16000] fp8e3 (e3m4): W shard, block-major packed
         wt[p, j*2000 + k*500 + n] = S * W[c*4000 + j*500 + n, k*128 + p]
         e3m4 halves the W stream vs fp16; with rt kept fp16 the logits
         rel-err is ~1.3e-2 (gate 2e-2). S=128 maps W into e3m4's
         normal range.
  - out  [32, 4000]  fp16 logits shard (host upcasts, adds b)
The stream is issued as 8 DMAs alternating the two HWDGE queues, with
the final block split fine so the kernel tail starts early.
"""

import numpy as np
from contextlib import ExitStack

B, T, D, V = 32, 2048, 512, 32000
NCORES = 8
VS = V // NCORES        # 4000 vocab columns per core
NUM_SLOTS, KP = 8192, 4
SEED = np.uint32(1234)
GOLD = np.uint32(0x9E3779B9)
KC = D // 128           # 4 contraction chunks
NBW = 500               # columns per block (one PSUM bank of fp32)
NB = VS // NBW          # 8 blocks per core
BLK = KC * NBW          # 2000 free elems per block in wt layout

# W-stream dtype: "f8" = e3m4 (1 byte, rel-err ~1.3e-2),
# "f16" = float16 (2 bytes, rel-err ~3e-4).
W_MODE = "f8"
WSCALE = 128.0
NWARM = 5               # PE HAM warm-up dummy matmuls

_prog_cache = {}
LAST_RESULTS = None     # stashed BassKernelResults (for profiling in test.py)


def _mix32(h):
    h = h.astype(np.uint32, copy=False)
    h = h ^ (h >> np.uint32(16))
    h = h * np.uint32(0x85EBCA6B)
    h = h ^ (h >> np.uint32(13))
    h = h * np.uint32(0xC2B2AE35)
    h = h ^ (h >> np.uint32(16))
    return h


def _probe_slots(tok):
    hx = _mix32(tok.astype(np.uint32) ^ SEED)
    offs = np.arange(KP, dtype=np.uint32) * GOLD
    return (_mix32(hx[..., None] + offs) % np.uint32(NUM_SLOTS)).astype(np.int32)


def _split_multi_waits(nc, limit=1):
    """The nix-baked walrus rejects instructions with more than `limit`
    sem-waits ("Too many sync wait commands", CoreV3GenImpl setupSyncWait).
    Hoist extra waits onto single-wait NOPs preceding the instruction on
    the same engine (waiting earlier on the same engine is always safe)."""
    import concourse.mybir as mybir

    for fn in nc.m.functions:
        for bb in fn.blocks:
            new_insts = []
            for ins in bb.instructions:
                si = ins.sync_info
                if si is not None and len(si.on_wait) > limit:
                    waits = list(si.on_wait)
                    extra, keep = waits[:-limit], waits[-limit:]
                    for idx, w in enumerate(extra):
                        new_insts.append(mybir.InstNoOp(
                            name=f"{ins.name}-wsplit{idx}",
                            sync_info=mybir.SyncInfo(on_wait=[w], on_update=[]),
                            bass_nofuse=True,
                            engine=ins.engine,
                        ))
                    ins.sync_info = mybir.SyncInfo(
                        on_wait=keep, on_update=list(si.on_update))
                new_insts.append(ins)
            bb.instructions[:] = new_insts


def _strip_entry_barrier(nc):
    """Remove the entry-BB all-engine boot barrier and the const-tile
    memsets (walrus flags those consts as having no readers). The barrier
    only serializes engine boot: every real dependency in the body is
    carried by Tile-generated semaphores, and the event-semaphore
    barrier instances are self-resetting, so the exit barriers are
    unaffected. This lets each engine (notably the DMA-trigger engines)
    start its body work as soon as it boots instead of waiting ~3us for
    the slowest engine."""
    import concourse.mybir as mybir

    def _is_barrier(ins):
        if not isinstance(ins, (mybir.InstDrain, mybir.InstEventSemaphore)):
            return False
        si = ins.sync_info
        names = [w.ant_name for w in (si.on_wait if si else [])]
        names += [getattr(u, "ant_name", "") or ""
                  for u in (si.on_update if si else [])]
        return any(n.startswith("barrier_") for n in names) or not names

    bb = nc.m.functions[0].blocks[0]
    bb.instructions[:] = [
        ins for ins in bb.instructions
        if not (isinstance(ins, mybir.InstMemset) or _is_barrier(ins))
    ]


def _build(wdt, split=True):
    import concourse.bass as bass
    import concourse.mybir as mybir
    from concourse.bass import MemorySpace
    from concourse.tile import TileContext

    f32 = mybir.dt.float32
    f16 = mybir.dt.float16
    fw = mybir.dt.float8e3 if wdt == "f8" else mybir.dt.float16

    nc = bass.Bass(monotonic_sem_count=0, enable_partition_id=False)
    rt = nc.declare_dram_parameter("rt", [128, KC * B], f16, isOutput=False)
    wt = nc.declare_dram_parameter("wt", [128, KC * VS], fw, isOutput=False)
    out = nc.declare_dram_parameter("out", [B, VS], f16, isOutput=True)

    # W-stream chunk plan: (free-dim start, length, engine idx 0=sync
    # 1=scalar). Blocks are consumed in pairs (col-group tiling), so the
    # leading pair is split at k-granularity to start the first matmuls
    # as early as possible; middle pairs are one 512 KB transfer each;
    # the trailing block is split fine (k01 | k2 | k3) so the kernel
    # tail (last matmul -> copy -> store) starts early.
    chunks = [
        (0 * BLK, 1 * BLK, 0),             # blk0
        (1 * BLK, 1 * BLK, 1),             # blk1
        (2 * BLK, 2 * BLK, 0),             # blks 2,3
        (4 * BLK, 2 * BLK, 1),             # blks 4,5
        (6 * BLK, 1 * BLK, 0),             # blk6
        (7 * BLK, 2 * NBW, 1),             # blk7 k01
        (7 * BLK + 2 * NBW, NBW, 0),       # blk7 k2
        (7 * BLK + 3 * NBW, NBW, 1),       # blk7 k3
    ]

    with TileContext(nc) as tc:
        with ExitStack() as ctx:
            const = ctx.enter_context(tc.tile_pool(name="const", bufs=1))
            rt_sb = const.tile([128, KC * B], f16)
            dumw = const.tile([128, 544], fw, name="dumw")

            wtp = ctx.enter_context(tc.tile_pool(name="wtp", bufs=len(chunks)))
            obuf = ctx.enter_context(tc.tile_pool(name="obuf", bufs=NB))
            with tc.tile_pool(name="mpsum", bufs=8, space=MemorySpace.PSUM) as mpsum:
                # rt load first on scalar (it gates the first LDWEIGHTS);
                # it is tiny so it barely delays the W stream behind it.
                nc.scalar.dma_start(rt_sb[:], rt[:])

                dma_engs = [nc.sync, nc.scalar]
                wq = []
                for (off, ln, ei) in chunks:
                    t = wtp.tile([128, ln], fw, name="wq")
                    dma_engs[ei].dma_start(t[:], wt[:, off:off + ln])
                    wq.append((off, ln, t))

                def moving(j, k):
                    g = j * BLK + k * NBW
                    for (off, ln, t) in wq:
                        if off <= g and g + NBW <= off + ln:
                            return t[:, g - off:g - off + NBW]
                    raise AssertionError("no chunk covers block")

                # PE warm-up: the HAM clock gate keeps the PE at 1.2 GHz
                # until it has seen ~3.4us of sustained matmul activity.
                # Dummy matmuls on an SBUF-garbage-free memset tile keep
                # the PE busy while the first W chunks are in flight so
                # the real matmuls run at 2.4 GHz.
                nc.vector.memset(dumw[:], 0.0)
                dps = mpsum.tile([32, 512], f32, name="ps")
                for _ in range(NWARM):
                    nc.tensor.matmul(
                        dps[:], dumw[:, :32], dumw[:, 32:544],
                        start=True, stop=True)

                # Blocks are processed in pairs on the two 32-column
                # groups of the PE array (col-group tiling): even block
                # -> col-group 0 / PSUM partitions [0:32], odd block ->
                # col-group 1 / PSUM partitions [32:64]. The two moving
                # streams run concurrently, halving the PE-serial time.
                ps_t, ob_t = [], []
                for j in range(NB):
                    ps = mpsum.tile([64, NBW], f32, name="ps")
                    ob = obuf.tile([64, NBW], f16, name="ob")
                    ps_t.append(ps)
                    ob_t.append(ob)

                def psum_ap(j):
                    lo = 32 * (j % 2)
                    return ps_t[j][lo:lo + B]

                for p in range(NB // 2):
                    for k in range(KC):
                        for j in (2 * p, 2 * p + 1):
                            nc.tensor.matmul(
                                psum_ap(j),
                                rt_sb[:, k * B:(k + 1) * B],
                                moving(j, k),
                                start=(k == 0),
                                stop=(k == KC - 1),
                                tile_position=(0, 32 * (j % 2)),
                            )

                    for j in (2 * p, 2 * p + 1):
                        lo = 32 * (j % 2)
                        ob = ob_t[j][lo:lo + B]
                        cols = slice(j * NBW, (j + 1) * NBW)
                        if j == NB - 1:
                            # Final block: halve the copy across DVE+ACT
                            # and store the halves on the two HWDGE
                            # queues so the last write receipt (which
                            # gates the kernel tail) lands sooner.
                            h = NBW // 2
                            nc.vector.tensor_copy(
                                ob[:, :h], psum_ap(j)[:, :h])
                            nc.scalar.copy(
                                ob[:, h:], psum_ap(j)[:, h:])
                            nc.sync.dma_start(
                                out[:, j * NBW:j * NBW + h], ob[:, :h])
                            nc.scalar.dma_start(
                                out[:, j * NBW + h:(j + 1) * NBW],
                                ob[:, h:])
                        else:
                            if j % 2 == 0:
                                nc.vector.tensor_copy(ob[:], psum_ap(j))
                            else:
                                nc.scalar.copy(ob[:], psum_ap(j))
                            # Stores: early even blocks on the SWDGE
                            # queue (keeps the HWDGE W-stream queues'
                            # FIFO completion order clean); later ones
                            # on whichever HWDGE queue has drained its
                            # share of the W stream.
                            store_eng = {0: nc.gpsimd, 2: nc.gpsimd,
                                         4: nc.gpsimd, 1: nc.sync,
                                         3: nc.sync, 5: nc.scalar,
                                         6: nc.scalar}[j]
                            store_eng.dma_start(out[:, cols], ob[:])
    if split:
        _split_multi_waits(nc)
        _strip_entry_barrier(nc)
    return nc


def _get_prog():
    if W_MODE not in _prog_cache:
        _prog_cache[W_MODE] = _build(W_MODE)
    return _prog_cache[W_MODE]


def _host_r(x, emb_table):
    """Integer hash/match preprocessing -> read vector r [B, D]."""
    ts = np.arange(0, T - 1, 2)
    ts = ts[ts + 1 < T - 1]                      # [P]
    wslots = _probe_slots(x[:, ts])              # [B, P, K]
    qslots = _probe_slots(x[:, -1])              # [B, K]
    m = (wslots[:, :, None, :] == qslots[:, None, :, None]).sum(
        axis=(2, 3), dtype=np.int32)             # [B, P]
    bs, ps = np.nonzero(m)
    r = np.zeros((B, D), np.float64)
    vtok = x[:, ts + 1]
    for bi, pi in zip(bs, ps):
        r[bi] += (m[bi, pi] / KP) * emb_table[vtok[bi, pi]].astype(np.float64)
    return r


def _pack_w(W):
    """[V, D] -> per-core block-major stream layout [NCORES, 128, KC*VS]."""
    import ml_dtypes
    np_w = ml_dtypes.float8_e3m4 if W_MODE == "f8" else np.float16
    Wq = (W.astype(np.float32) * np.float32(WSCALE)).astype(np_w)
    A = Wq.reshape(NCORES, NB, NBW, KC, 128)     # [c, j, n, k, p]
    return np.ascontiguousarray(
        A.transpose(0, 4, 1, 3, 2)).reshape(NCORES, 128, KC * VS)


def kernel(x, emb_table, W, b):
    global LAST_RESULTS
    from concourse.bass_utils import run_bass_kernel_spmd

    x = np.asarray(x)
    emb_table = np.asarray(emb_table, np.float32)
    W = np.asarray(W, np.float32)
    b = np.asarray(b, np.float32)

    r = _host_r(x, emb_table)                    # [B, D] float64
    rt = (r / WSCALE).astype(np.float16)         # fold W scale into rt
    rt_dev = np.ascontiguousarray(
        rt.T.reshape(KC, 128, B).transpose(1, 0, 2)).reshape(128, KC * B)
    wt_dev = _pack_w(W)

    nc = _get_prog()
    in_maps = [{"rt": rt_dev, "wt": wt_dev[c]} for c in range(NCORES)]

    res = None
    for attempt in range(3):
        try:
            res = run_bass_kernel_spmd(
                nc, in_maps, core_ids=list(range(NCORES)))
            break
        except Exception:
            # The axon-tunneled device occasionally reports a transient
            # NRT_EXEC_UNIT_UNRECOVERABLE on back-to-back NEFF loads;
            # a re-dispatch on the next attempt succeeds.
            if attempt == 2:
                raise
            import time
            time.sleep(2.0)
    LAST_RESULTS = res

    logits = np.empty((B, V), np.float32)
    for c in range(NCORES):
        logits[:, c * VS:(c + 1) * VS] = res.results[c]["out"]
    if np.any(b):
        logits += b[None, :]
    return logits


# revision 30
# speedup vs baseline: 1.1202x; 1.0896x over previous
"""Trainium2 Bass kernel for nn_BBPMAssociativeModel.

Model: per-batch associative memory — pairs (key, value-token) from the
input sequence are scatter-added into a 8192-slot memory via 4 hash
probes, the memory is read back at the query token's 4 probe slots,
and the mean read vector goes through a [D, V] classifier.

Algebraic collapse: the memory is never materialized, and the read
vector r is computed ON HOST (it is a tiny [B, D] combination of a few
embedding rows selected by integer hash matches — ~2 MFLOP):
    r_b = sum_p (m_{b,p} / K) * emb_table[x[b, 2p+1]]
The device does only the memory-bound classifier matmul
    logits = r @ W.T          (vocab-sharded over 8 cores)

Per-core device program (vocab shard of 4000 columns):
  - rt   [128, 128]  fp16: rT k-chunk-packed, rt[p, k*32+b] = r[b, k*128+p]/S
  - wt   [128, 16000] fp8e3 (e3m4): W shard, block-major packed
         wt[p, j*2000 + k*500 + n] = S * W[c*4000 + j*500 + n, k*128 + p]
         e3m4 halves the W stream vs fp16; with rt kept fp16 the logits
         rel-err is ~1.3e-2 (gate 2e-2). S=128 maps W into e3m4's
         normal range.
  - out  [32, 4000]  fp16 logits shard (host upcasts, adds b)
The stream is issued as 8 DMAs alternating the two HWDGE queues, with
the final block split fine so the kernel tail starts early.
"""

import numpy as np
from contextlib import ExitStack

B, T, D, V = 32, 2048, 512, 32000
NCORES = 8
VS = V // NCORES        # 4000 vocab columns per core
NUM_SLOTS, KP = 8192, 4
SEED = np.uint32(1234)
GOLD = np.uint32(0x9E3779B9)
KC = D // 128           # 4 contraction chunks
NBW = 500               # columns per block (one PSUM bank of fp32)
NB = VS // NBW          # 8 blocks per core
BLK = KC * NBW          # 2000 free elems per block in wt layout

# W-stream dtype: "f8" = e3m4 (1 byte, rel-err ~1.3e-2),
# "f16" = float16 (2 bytes, rel-err ~3e-4).
W_MODE = "f8"
WSCALE = 128.0
# PE HAM warm-up dummy matmuls. Over-provisioned on purpose: chunk
# arrival time varies run to run, and any PE-idle gap between warm-up
# and the first data-gated matmul resets the HAM busy-window
# qualification, leaving the whole kernel at 1.2 GHz (~3us slower).
# Excess warm-ups cost ~0.2us each once warm; a cold kernel costs ~3us.
NWARM = 7

_prog_cache = {}
LAST_RESULTS = None     # stashed BassKernelResults (for profiling in test.py)


def _mix32(h):
    h = h.astype(np.uint32, copy=False)
    h = h ^ (h >> np.uint32(16))
    h = h * np.uint32(0x85EBCA6B)
    h = h ^ (h >> np.uint32(13))
    h = h * np.uint32(0xC2B2AE35)
    h = h ^ (h >> np.uint32(16))
    return h


def _probe_slots(tok):
    hx = _mix32(tok.astype(np.uint32) ^ SEED)
    offs = np.arange(KP, dtype=np.uint32) * GOLD
    return (_mix32(hx[..., None] + offs) % np.uint32(NUM_SLOTS)).astype(np.int32)


def _split_multi_waits(nc, limit=1):
    """The nix-baked walrus rejects instructions with more than `limit`
    sem-waits ("Too many sync wait commands", CoreV3GenImpl setupSyncWait).
    Hoist extra waits onto single-wait NOPs preceding the instruction on
    the same engine (waiting earlier on the same engine is always safe)."""
    import concourse.mybir as mybir

    for fn in nc.m.functions:
        for bb in fn.blocks:
            new_insts = []
            for ins in bb.instructions:
                si = ins.sync_info
                if si is not None and len(si.on_wait) > limit:
                    waits = list(si.on_wait)
                    extra, keep = waits[:-limit], waits[-limit:]
                    for idx, w in enumerate(extra):
                        new_insts.append(mybir.InstNoOp(
                            name=f"{ins.name}-wsplit{idx}",
                            sync_info=mybir.SyncInfo(on_wait=[w], on_update=[]),
                            bass_nofuse=True,
                            engine=ins.engine,
                        ))
                    ins.sync_info = mybir.SyncInfo(
                        on_wait=keep, on_update=list(si.on_update))
                new_insts.append(ins)
            bb.instructions[:] = new_insts


def _strip_entry_barrier(nc):
    """Remove the entry-BB all-engine boot barrier and the const-tile
    memsets (walrus flags those consts as having no readers). The barrier
    only serializes engine boot: every real dependency in the body is
    carried by Tile-generated semaphores, and the event-semaphore
    barrier instances are self-resetting, so the exit barriers are
    unaffected. This lets each engine (notably the DMA-trigger engines)
    start its body work as soon as it boots instead of waiting ~3us for
    the slowest engine."""
    import concourse.mybir as mybir

    def _is_barrier(ins):
        if not isinstance(ins, (mybir.InstDrain, mybir.InstEventSemaphore)):
            return False
        si = ins.sync_info
        names = [w.ant_name for w in (si.on_wait if si else [])]
        names += [getattr(u, "ant_name", "") or ""
                  for u in (si.on_update if si else [])]
        return any(n.startswith("barrier_") for n in names) or not names

    bb = nc.m.functions[0].blocks[0]
    bb.instructions[:] = [
        ins for ins in bb.instructions
        if not (isinstance(ins, mybir.InstMemset) or _is_barrier(ins))
    ]


def _strip_second_exit_barrier(nc):
    """The exit BB runs TWO cross-engine barriers: one before the Pool
    engine's event-semaphore range-clear (needed: the clear must not race
    live Tile semaphore waits) and one after it (only needed when the
    NEFF body is re-entered in a loop; our kernel executes once per
    dispatch). Dropping the second barrier lets every engine reach its
    final NOTIFY ~0.6us earlier."""
    import concourse.mybir as mybir

    bb = nc.m.functions[0].blocks[-1]
    isa_idx = max(
        (i for i, ins in enumerate(bb.instructions)
         if isinstance(ins, mybir.InstISA)),
        default=None,
    )
    if isa_idx is None:
        return

    def _is_barrier2(ins):
        if not isinstance(ins, (mybir.InstDrain, mybir.InstEventSemaphore)):
            return False
        si = ins.sync_info
        names = [w.ant_name for w in (si.on_wait if si else [])]
        names += [getattr(u, "ant_name", "") or ""
                  for u in (si.on_update if si else [])]
        return any(n.startswith("barrier_") for n in names)

    tail = [ins for ins in bb.instructions[isa_idx + 1:]
            if not _is_barrier2(ins)]
    bb.instructions[isa_idx + 1:] = tail


def _build(wdt, split=True):
    import concourse.bass as bass
    import concourse.mybir as mybir
    from concourse.bass import MemorySpace
    from concourse.tile import TileContext

    f32 = mybir.dt.float32
    f16 = mybir.dt.float16
    fw = mybir.dt.float8e3 if wdt == "f8" else mybir.dt.float16

    # rt rides as raw fp16 bytes in the first 256 bytes of each wt
    # partition row (bitcast view on SBUF) — a separate tiny rt DMA
    # would either stall an HWDGE ring with sub-512B descriptors or sit
    # behind the SWDGE queue's 1.5-3us variable start latency, gating
    # the first matmul.
    RTB = 2 * KC * B if wdt == "f8" else KC * B

    nc = bass.Bass(monotonic_sem_count=0, enable_partition_id=False)
    wt = nc.declare_dram_parameter(
        "wt", [128, RTB + KC * VS], fw, isOutput=False)
    # out is block-major [j*B + b, n] so each block pair stores as one
    # contiguous [64, 500] DMA; the host un-permutes.
    out = nc.declare_dram_parameter("out", [NB * B, NBW], f16, isOutput=True)

    # W-stream chunk plan: (wt free-dim start, length, engine idx 0=sync
    # 1=scalar). Chunk 0 (rt bytes + pair 0) leads on the scalar HWDGE
    # queue, whose engine reaches its body earliest; pairs complete in
    # consumption order across the two queues. The trailing block is
    # split (k01 | k2 | k3) so the kernel tail starts early.
    chunks = [
        (0, RTB + 2 * BLK, 1),                   # rt + blks 0,1
        (RTB + 2 * BLK, 2 * BLK, 0),             # blks 2,3
        (RTB + 4 * BLK, 2 * BLK, 1),             # blks 4,5
        (RTB + 6 * BLK, BLK + 2 * NBW, 0),       # blk6 + blk7 k01
        (RTB + 7 * BLK + 2 * NBW, NBW, 0),       # blk7 k2
        (RTB + 7 * BLK + 3 * NBW, NBW, 1),       # blk7 k3
    ]

    with TileContext(nc) as tc:
        with ExitStack() as ctx:
            const = ctx.enter_context(tc.tile_pool(name="const", bufs=1))
            dumw = const.tile([128, 544], fw, name="dumw")

            wtp = ctx.enter_context(tc.tile_pool(name="wtp", bufs=len(chunks)))
            obuf = ctx.enter_context(tc.tile_pool(name="obuf", bufs=NB // 2))
            with tc.tile_pool(name="mpsum", bufs=6, space=MemorySpace.PSUM) as mpsum:
                dma_engs = [nc.sync, nc.scalar]
                wq = []
                for (off, ln, ei) in chunks:
                    t = wtp.tile([128, ln], fw, name="wq")
                    dma_engs[ei].dma_start(t[:], wt[:, off:off + ln])
                    wq.append((off, ln, t))

                rt_sb = wq[0][2][:, :RTB].bitcast(f16)   # [128, KC*B]

                def moving(j, k):
                    g = RTB + j * BLK + k * NBW
                    for (off, ln, t) in wq:
                        if off <= g and g + NBW <= off + ln:
                            return t[:, g - off:g - off + NBW]
                    raise AssertionError("no chunk covers block")

                # PE warm-up: the HAM clock gate keeps the PE at 1.2 GHz
                # until it has seen ~3.4us of sustained matmul activity.
                # Dummy matmuls on an SBUF-garbage-free memset tile keep
                # the PE busy while the first W chunks are in flight so
                # the real matmuls run at 2.4 GHz.
                nc.vector.memset(dumw[:], 0.0)
                dps = mpsum.tile([32, 512], f32, name="ps")
                for _ in range(NWARM):
                    nc.tensor.matmul(
                        dps[:], dumw[:, :32], dumw[:, 32:544],
                        start=True, stop=True)

                # Blocks are processed in pairs on the two 32-column
                # groups of the PE array (col-group tiling): even block
                # -> col-group 0 / PSUM partitions [0:32], odd block ->
                # col-group 1 / PSUM partitions [32:64]. The two moving
                # streams run concurrently, halving the PE-serial time.
                # Each pair shares one PSUM bank and one SBUF out tile
                # (subtile deps keep the two accumulation chains
                # independent), so a pair stores as ONE 64 KB DMA whose
                # DRAM side is the rearranged view of out below.
                # All stores ride the HWDGE queues: keeping the Pool
                # engine DMA-free makes its exit-block queue drain
                # (which gates the final cross-engine barrier) instant.
                outv = out
                NP = NB // 2
                n_store = 0
                for p in range(NP):
                    ps = mpsum.tile([2 * B, NBW], f32, name="ps")
                    ob = obuf.tile([2 * B, NBW], f16, name="ob")
                    last = p == NP - 1
                    # Final pair: run the k-rounds per column half so
                    # its copies and stores overlap its own matmuls and
                    # the last write receipt lands sooner.
                    halves = ((0, NBW // 2), (NBW // 2, NBW)) if last \
                        else ((0, NBW),)
                    for (h0, h1) in halves:
                        for k in range(KC):
                            for g, j in ((0, 2 * p), (1, 2 * p + 1)):
                                nc.tensor.matmul(
                                    ps[32 * g:32 * g + B, h0:h1],
                                    rt_sb[:, k * B:(k + 1) * B],
                                    moving(j, k)[:, h0:h1],
                                    start=(k == 0),
                                    stop=(k == KC - 1),
                                    tile_position=(0, 32 * g),
                                )
                        nc.vector.tensor_copy(
                            ob[:B, h0:h1], ps[:B, h0:h1])
                        nc.scalar.copy(
                            ob[B:, h0:h1], ps[B:, h0:h1])
                        rows = slice(2 * p * B, (2 * p + 2) * B)
                        store_eng = [nc.sync, nc.scalar][n_store % 2]
                        n_store += 1
                        store_eng.dma_start(
                            outv[rows, h0:h1], ob[:, h0:h1])
    if split:
        _split_multi_waits(nc)
        _strip_entry_barrier(nc)
        _strip_second_exit_barrier(nc)
    return nc


def _get_prog():
    if W_MODE not in _prog_cache:
        _prog_cache[W_MODE] = _build(W_MODE)
    return _prog_cache[W_MODE]


def _host_r(x, emb_table):
    """Integer hash/match preprocessing -> read vector r [B, D]."""
    ts = np.arange(0, T - 1, 2)
    ts = ts[ts + 1 < T - 1]                      # [P]
    wslots = _probe_slots(x[:, ts])              # [B, P, K]
    qslots = _probe_slots(x[:, -1])              # [B, K]
    m = (wslots[:, :, None, :] == qslots[:, None, :, None]).sum(
        axis=(2, 3), dtype=np.int32)             # [B, P]
    bs, ps = np.nonzero(m)
    r = np.zeros((B, D), np.float64)
    vtok = x[:, ts + 1]
    for bi, pi in zip(bs, ps):
        r[bi] += (m[bi, pi] / KP) * emb_table[vtok[bi, pi]].astype(np.float64)
    return r


def _pack_w(W):
    """[V, D] -> per-core block-major stream layout [NCORES, 128, KC*VS]."""
    import ml_dtypes
    np_w = ml_dtypes.float8_e3m4 if W_MODE == "f8" else np.float16
    Wq = (W.astype(np.float32) * np.float32(WSCALE)).astype(np_w)
    A = Wq.reshape(NCORES, NB, NBW, KC, 128)     # [c, j, n, k, p]
    return np.ascontiguousarray(
        A.transpose(0, 4, 1, 3, 2)).reshape(NCORES, 128, KC * VS)


def kernel(x, emb_table, W, b):
    global LAST_RESULTS
    from concourse.bass_utils import run_bass_kernel_spmd

    x = np.asarray(x)
    emb_table = np.asarray(emb_table, np.float32)
    W = np.asarray(W, np.float32)
    b = np.asarray(b, np.float32)

    r = _host_r(x, emb_table)                    # [B, D] float64
    rt = (r / WSCALE).astype(np.float16)         # fold W scale into rt
    rt_dev = np.ascontiguousarray(
        rt.T.reshape(KC, 128, B).transpose(1, 0, 2)).reshape(128, KC * B)
    wt_dev = _pack_w(W)
    import ml_dtypes
    np_w = ml_dtypes.float8_e3m4 if W_MODE == "f8" else np.float16
    rt_bytes = rt_dev.view(np.uint8)             # [128, 256]

    nc = _get_prog()
    in_maps = [
        {"wt": np.ascontiguousarray(np.concatenate(
            [rt_bytes, wt_dev[c].view(np.uint8)], axis=1)).view(np_w)}
        for c in range(NCORES)
    ]

    res = None
    for attempt in range(3):
        try:
            res = run_bass_kernel_spmd(
                nc, in_maps, core_ids=list(range(NCORES)))
            break
        except Exception:
            # The axon-tunneled device occasionally reports a transient
            # NRT_EXEC_UNIT_UNRECOVERABLE on back-to-back NEFF loads;
            # a re-dispatch on the next attempt succeeds.
            if attempt == 2:
                raise
            import time
            time.sleep(2.0)
    LAST_RESULTS = res

    logits = np.empty((B, V), np.float32)
    for c in range(NCORES):
        blk = res.results[c]["out"].reshape(NB, B, NBW)      # [j, b, n]
        logits[:, c * VS:(c + 1) * VS] = (
            blk.transpose(1, 0, 2).reshape(B, VS))
    if np.any(b):
        logits += b[None, :]
    return logits
